# revision 60
# baseline (speedup 1.0000x reference)
"""Trainium2 Bass kernel for nn_AttentionLayer (B=4, S=2048, H=12, D=64).

Sharding: 8 cores = 4 batches x 2 head-groups (6 heads each).
Per core: QKV projections for its 384 W-columns, then per-(head) attention
with a UniLM prefix "staircase" mask (cumsum of segment_ids is
non-decreasing, so each query attends to a prefix of keys). Fully-masked
[128k x 512q] tiles are skipped at program-build time (union over the 4
batches, so one SPMD program serves all cores); partially-masked tiles get
a multiplicative 0/1 mask after exp, and only their visible q-suffix is
computed at all (scores, exp, mask and ctx all start at the tile's q0).

Layout: scores are computed transposed (k on partitions, q free) so probs
feed the ctx matmul as the moving operand with v stationary. A ones-column
in v accumulates the softmax denominator for free.

Performance structure (what made this fast, vs the naive phased version):
- The steady-state bottleneck is the Scalar engine's exp stream, so the
  whole kernel is paced by ACT. The PE work per softmax group (~1.3us) is
  less than the exp time (~2.2us), so projection matmuls are emitted as
  filler "quanta" INSIDE the attention loop: the PE never idles long
  enough for the HAM clock gate to re-throttle it to 1.2 GHz (which
  previously doubled all matmul times), and the projection phase costs no
  extra wall-clock.
- ctx matmuls are emitted one group BEHIND their scores (lag-1), so the
  PE FIFO head never blocks on a pending exp while independent scores /
  projection matmuls wait behind it.
- ctx matmuls for qb>=1 run in fp8e4 DoubleRow mode: one matmul contracts
  a PAIR of k-chunks (256 virtual rows), halving the ctx matmul count.
  qb=0 (few visible keys, fp8 noise does not average out) keeps a bf16
  path with a small bf16 copy of the v chunks it needs.
- Softmax normalization runs entirely off the PE: fast approximate
  reciprocal (DVE) + partition broadcast (GPSIMD ucode) + one fused
  PSUM*bcast multiply (DVE). Output DMAs go per head-pair.
"""

import sys

if "/opt/trn_rl_repo" not in sys.path:
    sys.path.insert(0, "/opt/trn_rl_repo")

from contextlib import ExitStack

import ml_dtypes
import numpy as np

import concourse.bass as bass
import concourse.mybir as mybir
import concourse.tile as tile
from concourse import bacc, library_config
from concourse.bass_utils import run_bass_kernel_spmd

B, S, W, H, D = 4, 2048, 768, 12, 64
NCORES = 8
HPC = 6  # heads per core
QB = 512  # q block (free dim of a scores tile)
KC = 128  # k chunk (partition dim of a scores tile)
NQB = S // QB
NKC = S // KC
MC = 3  # 128-row chunks of the 384 per-core W-columns
FKC = W // 128  # feature chunks (contraction for projections)
HD = HPC * D  # 384
VE = 80  # per-head pitch in v_aug (64 d + 1 ones + pad to 16B for DoubleRow)
VW = HPC * VE  # v_aug row width per k-chunk
ACT_GROUP = 2  # k-chunks per ACT instruction = one fp8 DoubleRow ctx pair

F32 = mybir.dt.float32
BF16 = mybir.dt.bfloat16
FP8 = mybir.dt.float8e4

TRACE = False  # set by test.py to profile
LAST_RESULTS = None  # BassKernelResults of the last run (for test.py)


def _ensure_ntff_hook():
    """This image's antenv lacks axon_hooks; register the ctypes NTFF
    profile hook from trn_agent_boot ourselves so trace=True works."""
    import types

    if "antenv.axon_hooks" in sys.modules:
        return
    try:
        from trn_agent_boot.trn_boot import _ntff_profile_via_ctypes

        hook = _ntff_profile_via_ctypes("/opt/axon/libaxon_pjrt.so")
    except Exception:
        hook = None
    mod = types.ModuleType("antenv.axon_hooks")
    mod._hook = hook
    mod.set_axon_ntff_profile_hook = lambda h: setattr(mod, "_hook", h)
    mod.get_axon_ntff_profile_hook = lambda: mod._hook
    sys.modules["antenv.axon_hooks"] = mod
    # artifact upload needs egress this sandbox doesn't have
    import concourse.bass_utils as _bu

    _bu.upload_artifacts = lambda d: "local://" + str(d)


def _classify(seg):
    """Union-over-batches tile classification from segment_ids.

    Returns (cumsums [B,S], per-qb visible k-chunk lists, boundary index,
    q-range maps). Element (k, q) is visible iff cs[k] <= cs[q]; cs is
    non-decreasing, so visibility per (kc, qb) tile is a q-suffix:
    q0 = first col with any visible element (union over batches),
    q1 = first col from which every batch sees the whole chunk.
    Cols < q0 need no compute at all; cols in [q0, q1) need the 0/1 mask.
    """
    cs = np.cumsum(np.asarray(seg, np.int64), axis=1)
    vis_lists = [[] for _ in range(NQB)]
    bnd_index = {}
    q0map = {}
    q1map = {}
    for qb in range(NQB):
        for kc in range(NKC):
            any_computed = False
            all_full_vis = True
            q0u, q1u = QB, 0
            for b in range(B):
                c = cs[b]
                full_mask = c[kc * KC] > c[qb * QB + QB - 1]
                full_vis = c[kc * KC + KC - 1] <= c[qb * QB]
                if not full_mask:
                    any_computed = True
                if not full_vis:
                    all_full_vis = False
                qcs = c[qb * QB : (qb + 1) * QB]
                anyv = np.nonzero(qcs >= c[kc * KC])[0]
                fullv = np.nonzero(qcs >= c[kc * KC + KC - 1])[0]
                q0u = min(q0u, int(anyv[0]) if len(anyv) else QB)
                q1u = max(q1u, int(fullv[0]) if len(fullv) else QB)
            if any_computed:
                vis_lists[qb].append(kc)
                if not all_full_vis:
                    bnd_index[(kc, qb)] = len(bnd_index)
                    q0map[(kc, qb)] = (q0u // 16) * 16
                    q1map[(kc, qb)] = min(QB, ((q1u + 15) // 16) * 16)
                else:
                    q0map[(kc, qb)] = 0
                    q1map[(kc, qb)] = 0
    return cs, vis_lists, bnd_index, (q0map, q1map)


def _build_program(vis_lists, bnd_index, qmaps):
    nc = bacc.Bacc()
    q0map, q1map = qmaps
    n_bnd = max(len(bnd_index), 1)
    # qb=0 queries attend very few keys, so fp8 ctx quantization noise does
    # not average out there (worst ~6e-2 rel vs ~3e-3 for qb>=1). Keep a
    # small bf16 copy of the v chunks qb=0 needs and run its ctx in bf16.
    N0 = len(vis_lists[0])
    VW16 = HPC * (D + 1)

    xT_d = nc.declare_dram_parameter("xT", [128, FKC * S], BF16, isOutput=False)
    wq_d = nc.declare_dram_parameter("wq", [128, FKC * HD], BF16, isOutput=False)
    wk_d = nc.declare_dram_parameter("wk", [128, FKC * HD], BF16, isOutput=False)
    wv_d = nc.declare_dram_parameter("wv", [128, FKC * HD], BF16, isOutput=False)
    bqk_d = nc.declare_dram_parameter("bqk", [128, 2 * MC], F32, isOutput=False)
    bvb_d = nc.declare_dram_parameter("bvb", [128, HD], F32, isOutput=False)
    csb_d = nc.declare_dram_parameter("cs_bcast", [128, S], F32, isOutput=False)
    csp_d = nc.declare_dram_parameter("cs_part", [128, NKC], F32, isOutput=False)
    out_d = nc.declare_dram_parameter("ctxT", [MC * 128, S], F32, isOutput=True)

    with ExitStack() as ctx:
        tc = ctx.enter_context(tile.TileContext(nc))
        persist = ctx.enter_context(tc.tile_pool(name="persist", bufs=1))

        qt = persist.tile([128, MC * S], BF16)
        kt = persist.tile([128, MC * S], BF16)
        v = persist.tile([128, NKC * VW], FP8)
        vb16 = persist.tile([128, N0 * VW16], BF16)
        ctxt = persist.tile([128, MC * S], F32)
        msk = persist.tile([128, n_bnd * QB], FP8)
        mskb = persist.tile([128, n_bnd * QB], BF16)
        cs_b = persist.tile([128, S], F32)
        cs_p = persist.tile([128, NKC], F32)
        bqk_sb = persist.tile([128, 2 * MC], F32)
        bv_sb = persist.tile([128, HD], F32)
        warmsrc = persist.tile([128, 640], BF16)
        nc.vector.memset(warmsrc, 0.0)
        nc.gpsimd.load_library(library_config.attn)  # partition_broadcast ucode
        nc.sync.dma_start(out=cs_b, in_=csb_d[:])
        nc.sync.dma_start(out=cs_p, in_=csp_d[:])
        nc.sync.dma_start(out=bqk_sb, in_=bqk_d[:])
        nc.sync.dma_start(out=bv_sb, in_=bvb_d[:])

        # 0/1 masks for boundary tiles, shared by all 6 heads of this core.
        # fp8 copy for the qb>=1 fp8 ctx path, bf16 copy for the qb=0 path.
        for (kc, qb), bi in bnd_index.items():
            dstm = mskb if qb == 0 else msk
            nc.vector.tensor_scalar(
                out=dstm[:, bi * QB : (bi + 1) * QB],
                in0=cs_b[:, qb * QB : (qb + 1) * QB],
                scalar1=cs_p[:, kc : kc + 1],
                scalar2=None,
                op0=mybir.AluOpType.is_ge,
            )

        with (
            tc.tile_pool(name="ld", bufs=1) as ld,
            tc.tile_pool(name="pps", bufs=2, space="PSUM") as pps,
            tc.tile_pool(name="scps", bufs=2, space="PSUM") as scps,
            tc.tile_pool(name="ctxps", bufs=2, space="PSUM") as ctxps,
            tc.tile_pool(name="expp", bufs=3) as expp,
            tc.tile_pool(name="lpool", bufs=4) as lpool,
        ):
            # wq/wk are laid out mc-major on the host so the mc=0 slices the
            # prelude needs land first; xt lands per feature-chunk so the
            # first projection matmuls start as soon as chunk 0 arrives.
            MB = FKC * 128  # one mc block of wq/wk columns
            xt = ld.tile([128, FKC * S], BF16)
            wq_sb = ld.tile([128, MC * MB], BF16)
            wk_sb = ld.tile([128, MC * MB], BF16)
            wv_sb = ld.tile([128, FKC * HD], BF16)
            # Startup-critical tensors go in halves: one dma_start lands on a
            # single HW queue, so splitting buys queue parallelism.
            hm = MB // 2
            nc.sync.dma_start(out=wq_sb[:, 0:hm], in_=wq_d[:, 0:hm])
            nc.sync.dma_start(out=wq_sb[:, hm:MB], in_=wq_d[:, hm:MB])
            nc.sync.dma_start(out=wk_sb[:, 0:hm], in_=wk_d[:, 0:hm])
            nc.sync.dma_start(out=wk_sb[:, hm:MB], in_=wk_d[:, hm:MB])
            for kc in range(FKC):
                h2 = kc * S + S // 2
                nc.sync.dma_start(out=xt[:, kc * S : h2], in_=xT_d[:, kc * S : h2])
                nc.sync.dma_start(out=xt[:, h2 : (kc + 1) * S], in_=xT_d[:, h2 : (kc + 1) * S])
            nc.sync.dma_start(out=wv_sb, in_=wv_d[:])
            nc.sync.dma_start(out=wq_sb[:, MB:], in_=wq_d[:, MB:])
            nc.sync.dma_start(out=wk_sb[:, MB:], in_=wk_d[:, MB:])

            v4 = v.rearrange("p (s h e) -> p s h e", h=HPC, e=VE)
            v_ones = v4[:, :, :, D : D + 1]
            nc.vector.memset(v_ones, 1.0)
            vb4 = vb16.rearrange("p (s h e) -> p s h e", h=HPC, e=D + 1)
            nc.vector.memset(vb4[:, :, :, D : D + 1], 1.0)

            # --- projection quanta: one PSUM bank accumulated over the 6
            # feature chunks, drained on DVE (ACT is the kernel bottleneck).
            def qk_quantum(pi, mc, nb, drain_on_act=False):
                w_sb = wq_sb if pi == 0 else wk_sb
                out_sb = qt if pi == 0 else kt
                ps = pps.tile([128, QB], F32, tag="proj", name="psqk")
                for kc in range(FKC):
                    nc.tensor.matmul(
                        ps,
                        lhsT=w_sb[:, mc * MB + kc * 128 : mc * MB + kc * 128 + 128],
                        rhs=xt[:, kc * S + nb * QB : kc * S + (nb + 1) * QB],
                        start=(kc == 0),
                        stop=(kc == FKC - 1),
                    )
                if drain_on_act:
                    # leisure-popped quanta (consumed much later): the drain
                    # doubles as filler for gaps in the exp dependency chain.
                    nc.scalar.activation(
                        out=out_sb[:, mc * S + nb * QB : mc * S + (nb + 1) * QB],
                        in_=ps,
                        func=mybir.ActivationFunctionType.Identity,
                        bias=bqk_sb[:, pi * MC + mc : pi * MC + mc + 1],
                        scale=1.0,
                    )
                else:
                    # demanded quanta (consumed within a few groups): drain on
                    # DVE so the result is not stuck behind queued exps in
                    # the ACT FIFO.
                    nc.vector.tensor_scalar_add(
                        out_sb[:, mc * S + nb * QB : mc * S + (nb + 1) * QB],
                        ps,
                        bqk_sb[:, pi * MC + mc : pi * MC + mc + 1],
                    )

            def v_quantum(sc):
                ps = pps.tile([128, HD], F32, tag="proj", name="psv")
                for kc in range(FKC):
                    nc.tensor.matmul(
                        ps,
                        lhsT=xt[:, kc * S + sc * KC : kc * S + sc * KC + KC],
                        rhs=wv_sb[:, kc * HD : (kc + 1) * HD],
                        start=(kc == 0),
                        stop=(kc == FKC - 1),
                    )
                nc.vector.tensor_add(
                    v4[:, sc, :, 0:D],
                    ps.rearrange("p (h e) -> p h e", e=D),
                    bv_sb.rearrange("p (h e) -> p h e", e=D),
                )
                if sc < N0:
                    nc.vector.tensor_add(
                        vb4[:, sc, :, 0:D],
                        ps.rearrange("p (h e) -> p h e", e=D),
                        bv_sb.rearrange("p (h e) -> p h e", e=D),
                    )

            # Projection work is emitted as PE filler spread across the
            # attention loop: emission order defines both the dependency DAG
            # and the scheduler priority, so each quantum must be emitted
            # before its first consumer (require(), with a 2-group lookahead)
            # and the slack is spread evenly (leisure pops) to keep PE duty
            # high everywhere — otherwise the HAM clock gate re-throttles
            # the PE to 1.2 GHz and every matmul doubles in cost.
            emitted = set()

            def emit_quantum(fq, drain_on_act=False):
                if fq in emitted:
                    return
                emitted.add(fq)
                if fq[0] == "v":
                    v_quantum(fq[1])
                else:
                    qk_quantum(fq[1], fq[2], fq[3], drain_on_act)

            filler = []
            for nb in range(1, NQB):
                filler.append(("qk", 0, 0, nb))
            for nb in range(2, NQB):
                filler.append(("qk", 1, 0, nb))
            for sc in range(N0, NKC):
                filler.append(("v", sc))
            for mc in (1, 2):
                for pi in range(2):
                    for nb in range(NQB):
                        filler.append(("qk", pi, mc, nb))

            demand_q = []

            def require(fq):
                # hard: the consumer is being emitted right now
                if fq in emitted:
                    return
                if fq in filler:
                    filler.remove(fq)
                if fq in demand_q:
                    demand_q.remove(fq)
                emit_quantum(fq)

            def require_soon(fq):
                # soft: needed within the next few groups; emitted by the
                # per-site popper at a capped one-quantum-per-site rate so
                # the PE work between consecutive scores groups never
                # exceeds the exp time (else ACT starves).
                if fq in emitted or fq in demand_q:
                    return
                if fq in filler:
                    filler.remove(fq)
                demand_q.append(fq)

            def warm_dummy():
                # Throwaway matmul that keeps the PE array active when no
                # real filler is left: sub-threshold PE duty makes the HAM
                # clock gate drop the PE to 1.2 GHz, which costs far more
                # than these extra 213ns matmuls.
                ps = pps.tile([128, QB], F32, tag="proj", name="warm")
                nc.tensor.matmul(
                    ps, lhsT=warmsrc[:, 0:128], rhs=warmsrc[:, 128:128 + QB],
                    start=True, stop=True,
                )

            # Warm the PE through the input-DMA wait so the prelude
            # projections run at 2.4 GHz instead of the cold 1.2 GHz.
            for _ in range(30):
                warm_dummy()

            def leisure_pop():
                if demand_q:
                    emit_quantum(demand_q.pop(0))
                elif filler:
                    emit_quantum(filler.pop(0), drain_on_act=True)
                else:
                    for _ in range(4):
                        warm_dummy()

            def group_needs(hp, qb, g):
                needs = [("qk", 0, hp, qb)]
                nb_hi = (g[-1] * KC + KC - 1) // QB
                for nb in range(nb_hi + 1):
                    needs.append(("qk", 1, hp, nb))
                for kc in g:
                    needs.append(("v", kc))
                return needs

            # Prelude: what the first two groups consume (the rest of the
            # v chunks for qb=0 are demand-pulled by the slot lookahead).
            emit_quantum(("qk", 0, 0, 0))
            emit_quantum(("qk", 1, 0, 0))
            for sc in range(2):
                emit_quantum(("v", sc))
            for sc in range(2, N0):
                filler.insert(sc - 2, ("v", sc))

            # --- attention ---
            def emit_scores_group(hp, qb, g, gq0, fp8_path):
                mcq = hp
                sps = {}
                esb = {}
                for par in range(2):
                    sps[par] = scps.tile(
                        [128, ACT_GROUP * QB], F32, tag="sps", name=f"sps{par}"
                    )
                    esb[par] = expp.tile(
                        [128, ACT_GROUP * QB],
                        FP8 if fp8_path else BF16,
                        tag="esb8" if fp8_path else "esb16",
                        name=f"esb{par}",
                    )
                for j, kc in enumerate(g):
                    for par in range(2):
                        po = par * 64
                        nc.tensor.matmul(
                            sps[par][:, j * QB + gq0 : (j + 1) * QB],
                            lhsT=kt[po : po + 64, mcq * S + kc * KC : mcq * S + kc * KC + KC],
                            rhs=qt[po : po + 64, mcq * S + qb * QB + gq0 : mcq * S + (qb + 1) * QB],
                            start=True,
                            stop=True,
                        )
                for par in range(2):
                    if len(g) == 2:
                        src = sps[par].rearrange("p (j q) -> p j q", j=2)[:, :, gq0:]
                        dst = esb[par].rearrange("p (j q) -> p j q", j=2)[:, :, gq0:]
                    else:
                        src = sps[par][:, gq0:QB]
                        dst = esb[par][:, gq0:QB]
                    nc.scalar.activation(
                        out=dst,
                        in_=src,
                        func=mybir.ActivationFunctionType.Exp,
                        scale=1.0 / float(np.sqrt(np.float32(D))),
                    )
                for j, kc in enumerate(g):
                    bi = bnd_index.get((kc, qb))
                    if bi is None:
                        continue
                    q1 = q1map[(kc, qb)]
                    srcm = msk if fp8_path else mskb
                    for par in range(2):
                        nc.vector.tensor_mul(
                            esb[par][:, j * QB + gq0 : j * QB + q1],
                            esb[par][:, j * QB + gq0 : j * QB + q1],
                            srcm[:, bi * QB + gq0 : bi * QB + q1],
                        )
                return esb

            def emit_ctx_group(hp, qb, g, gq0, fp8_path, esb, cps, unit, n_units):
                if fp8_path:
                    for par in range(2):
                        h = 2 * hp + par
                        if len(g) == 2:
                            nc.tensor.matmul(
                                cps[par][:, gq0:],
                                lhsT=v4[:, g[0] : g[0] + 2, h, 0 : D + 1],
                                rhs=esb[par].rearrange("p (j q) -> p j q", j=2)[:, :, gq0:],
                                start=(unit == 0),
                                stop=(unit == n_units - 1),
                                perf_mode=mybir.MatmulPerfMode.DoubleRow,
                            )
                        else:
                            nc.tensor.matmul(
                                cps[par][:, gq0:],
                                lhsT=v4[:, g[0], h, 0 : D + 1],
                                rhs=esb[par][:, gq0:QB],
                                start=(unit == 0),
                                stop=(unit == n_units - 1),
                            )
                    return unit + 1
                for j, kc in enumerate(g):
                    for par in range(2):
                        h = 2 * hp + par
                        nc.tensor.matmul(
                            cps[par][:, gq0:],
                            lhsT=vb16[:, kc * VW16 + h * (D + 1) : kc * VW16 + (h + 1) * (D + 1)],
                            rhs=esb[par][:, j * QB + gq0 : (j + 1) * QB],
                            start=(unit + j == 0),
                            stop=(unit + j == n_units - 1),
                        )
                return unit + len(g)

            all_slots = []
            for hp in range(HPC // 2):
                for qb in range(NQB):
                    vis = vis_lists[qb]
                    gs = [vis[i : i + ACT_GROUP] for i in range(0, len(vis), ACT_GROUP)]
                    for g in gs:
                        all_slots.append((hp, qb, g))
            si = 0
            for fut in all_slots[0:2]:
                for need in group_needs(*fut):
                    require(need)

            def emit_drain(hp_d, qb_d, cps_d):
                # drain: 1/l via fast reciprocal, broadcast over the 64
                # d-rows on the (idle) GPSIMD engine, then one fused
                # DVE multiply PSUM*bcast -> ctxt. Nothing lands in the
                # PE instruction stream, so matmuls never stall on it.
                # Called AFTER the next q-block's first scores group is
                # emitted, so the drain latency never gaps the exp stream.
                for par in range(2):
                    po = par * 64
                    # NB: the custom-DVE reciprocal misreads on HW when the
                    # input base partition differs from the output's, so
                    # stage l on partition 0 with a standard copy first.
                    lt = lpool.tile([1, QB], F32, tag="lt", name="lt")
                    lr = lpool.tile([1, QB], F32, tag="lr", name="lr")
                    bc = lpool.tile([64, QB], F32, tag="bc", name="bc")
                    nc.vector.tensor_copy(lt, cps_d[par][64:65, :])
                    nc.vector.reciprocal_approx_fast(out=lr, in_=lt)
                    nc.gpsimd.partition_broadcast(bc, lr)
                    nc.vector.tensor_mul(
                        ctxt[po : po + 64, hp_d * S + qb_d * QB : hp_d * S + (qb_d + 1) * QB],
                        cps_d[par][0:64, :],
                        bc,
                    )
                nc.sync.dma_start(
                    out=out_d[hp_d * 128 : (hp_d + 1) * 128, qb_d * QB : (qb_d + 1) * QB],
                    in_=ctxt[:, hp_d * S + qb_d * QB : hp_d * S + (qb_d + 1) * QB],
                )

            pending_drain = None
            for hp in range(HPC // 2):
                for qb in range(NQB):
                    vis = vis_lists[qb]
                    fp8_path = qb != 0
                    groups = [vis[i : i + ACT_GROUP] for i in range(0, len(vis), ACT_GROUP)]
                    n_units = len(groups) if fp8_path else len(vis)
                    cps = {}
                    for par in range(2):
                        cps[par] = ctxps.tile([65, QB], F32, tag="cps", name=f"cps{par}")
                    unit = 0
                    prev = None
                    for gi, g in enumerate(groups):
                        for need in group_needs(hp, qb, g):
                            require(need)
                        gq0 = min(q0map[(kc, qb)] for kc in g)
                        esb = emit_scores_group(hp, qb, g, gq0, fp8_path)
                        if gi == 0 and pending_drain is not None:
                            emit_drain(*pending_drain)
                            pending_drain = None
                        for fut in all_slots[si + 1 : si + 4]:
                            for need in group_needs(*fut):
                                require_soon(need)
                        leisure_pop()
                        si += 1
                        if prev is not None:
                            unit = emit_ctx_group(hp, qb, *prev, cps, unit, n_units)
                        prev = (g, gq0, fp8_path, esb)
                    leisure_pop()
                    unit = emit_ctx_group(hp, qb, *prev, cps, unit, n_units)
                    pending_drain = (hp, qb, cps)
            emit_drain(*pending_drain)

    nc.finalize()
    return nc


def _core_inputs(x, segment_ids, Wq, bq, Wk, bk, Wv, bv, cs, core):
    b, h0 = core // 2, HPC * (core % 2)
    cols = slice(h0 * D, (h0 + HPC) * D)
    xT = np.ascontiguousarray(x[b].T)  # [768, 2048]
    xT_s = (
        xT.reshape(FKC, 128, S).transpose(1, 0, 2).reshape(128, FKC * S)
    ).astype(ml_dtypes.bfloat16)

    def wprep(Wm, mc_major):
        ws = Wm[:, cols]  # [768, 384]
        if mc_major:  # [128, (mc, kc, 128)] so per-mc DMA chunks are contiguous
            arr = ws.reshape(FKC, 128, MC, 128).transpose(1, 2, 0, 3)
        else:  # [128, (kc, col)]
            arr = ws.reshape(FKC, 128, HD).transpose(1, 0, 2)
        return np.ascontiguousarray(arr.reshape(128, FKC * HD)).astype(
            ml_dtypes.bfloat16
        )

    bq_s = np.ascontiguousarray(bq[cols].reshape(MC, 128).T)
    bk_s = np.ascontiguousarray(bk[cols].reshape(MC, 128).T)
    bqk = np.concatenate([bq_s, bk_s], axis=1)  # [128, 6]
    bvb = np.ascontiguousarray(np.broadcast_to(bv[cols], (128, HD)))
    csf = cs[b].astype(np.float32)
    cs_bcast = np.ascontiguousarray(np.broadcast_to(csf, (128, S)))
    cs_part = np.ascontiguousarray(csf.reshape(NKC, KC).T)
    return {
        "xT": np.ascontiguousarray(xT_s),
        "wq": wprep(Wq, True),
        "wk": wprep(Wk, True),
        "wv": wprep(Wv, False),
        "bqk": np.ascontiguousarray(bqk),
        "bvb": bvb,
        "cs_bcast": cs_bcast,
        "cs_part": cs_part,
    }


def kernel(x, segment_ids, Wq, bq, Wk, bk, Wv, bv):
    global LAST_RESULTS
    x = np.asarray(x, np.float32)
    segment_ids = np.asarray(segment_ids)
    Wq, bq = np.asarray(Wq, np.float32), np.asarray(bq, np.float32)
    Wk, bk = np.asarray(Wk, np.float32), np.asarray(bk, np.float32)
    Wv, bv = np.asarray(Wv, np.float32), np.asarray(bv, np.float32)

    cs, vis_lists, bnd_index, qmaps = _classify(segment_ids)
    nc = _build_program(vis_lists, bnd_index, qmaps)
    in_maps = [
        _core_inputs(x, segment_ids, Wq, bq, Wk, bk, Wv, bv, cs, c)
        for c in range(NCORES)
    ]
    if TRACE:
        _ensure_ntff_hook()
    res = run_bass_kernel_spmd(nc, in_maps, list(range(NCORES)), trace=TRACE)
    LAST_RESULTS = res

    out = np.empty((B, S, W), np.float32)
    for c in range(NCORES):
        b, h0 = c // 2, HPC * (c % 2)
        out[b, :, h0 * D : (h0 + HPC) * D] = res.results[c]["ctxT"].T
    return out


# revision 61
# speedup vs baseline: 1.0089x; 1.0089x over previous
"""Trainium2 Bass kernel for nn_AttentionLayer (B=4, S=2048, H=12, D=64).

Sharding: 8 cores = 4 batches x 2 head-groups (6 heads each).
Per core: QKV projections for its 384 W-columns, then per-(head) attention
with a UniLM prefix "staircase" mask (cumsum of segment_ids is
non-decreasing, so each query attends to a prefix of keys). Fully-masked
[128k x 512q] tiles are skipped at program-build time (union over the 4
batches, so one SPMD program serves all cores); partially-masked tiles get
a multiplicative 0/1 mask after exp, and only their visible q-suffix is
computed at all (scores, exp, mask and ctx all start at the tile's q0).

Layout: scores are computed transposed (k on partitions, q free) so probs
feed the ctx matmul as the moving operand with v stationary. A ones-column
in v accumulates the softmax denominator for free.

Performance structure (what made this fast, vs the naive phased version):
- The steady-state bottleneck is the Scalar engine's exp stream, so the
  whole kernel is paced by ACT. The PE work per softmax group (~1.3us) is
  less than the exp time (~2.2us), so projection matmuls are emitted as
  filler "quanta" INSIDE the attention loop: the PE never idles long
  enough for the HAM clock gate to re-throttle it to 1.2 GHz (which
  previously doubled all matmul times), and the projection phase costs no
  extra wall-clock.
- ctx matmuls are emitted one group BEHIND their scores (lag-1), so the
  PE FIFO head never blocks on a pending exp while independent scores /
  projection matmuls wait behind it.
- ctx matmuls for qb>=1 run in fp8e4 DoubleRow mode: one matmul contracts
  a PAIR of k-chunks (256 virtual rows), halving the ctx matmul count.
  qb=0 (few visible keys, fp8 noise does not average out) keeps a bf16
  path with a small bf16 copy of the v chunks it needs.
- Softmax normalization runs entirely off the PE: fast approximate
  reciprocal (DVE) + partition broadcast (GPSIMD ucode) + one fused
  PSUM*bcast multiply (DVE). Output DMAs go per head-pair.
"""

import sys

if "/opt/trn_rl_repo" not in sys.path:
    sys.path.insert(0, "/opt/trn_rl_repo")

from contextlib import ExitStack

import ml_dtypes
import numpy as np

import concourse.bass as bass
import concourse.mybir as mybir
import concourse.tile as tile
from concourse import bacc, library_config
from concourse.bass_utils import run_bass_kernel_spmd

B, S, W, H, D = 4, 2048, 768, 12, 64
NCORES = 8
HPC = 6  # heads per core
QB = 512  # q block (free dim of a scores tile)
KC = 128  # k chunk (partition dim of a scores tile)
NQB = S // QB
NKC = S // KC
MC = 3  # 128-row chunks of the 384 per-core W-columns
FKC = W // 128  # feature chunks (contraction for projections)
HD = HPC * D  # 384
VE = 80  # per-head pitch in v_aug (64 d + 1 ones + pad to 16B for DoubleRow)
VW = HPC * VE  # v_aug row width per k-chunk
ACT_GROUP = 2  # k-chunks per ACT instruction = one fp8 DoubleRow ctx pair

F32 = mybir.dt.float32
BF16 = mybir.dt.bfloat16
FP8 = mybir.dt.float8e4

TRACE = False  # set by test.py to profile
LAST_RESULTS = None  # BassKernelResults of the last run (for test.py)


def _ensure_ntff_hook():
    """This image's antenv lacks axon_hooks; register the ctypes NTFF
    profile hook from trn_agent_boot ourselves so trace=True works."""
    import types

    if "antenv.axon_hooks" in sys.modules:
        return
    try:
        from trn_agent_boot.trn_boot import _ntff_profile_via_ctypes

        hook = _ntff_profile_via_ctypes("/opt/axon/libaxon_pjrt.so")
    except Exception:
        hook = None
    mod = types.ModuleType("antenv.axon_hooks")
    mod._hook = hook
    mod.set_axon_ntff_profile_hook = lambda h: setattr(mod, "_hook", h)
    mod.get_axon_ntff_profile_hook = lambda: mod._hook
    sys.modules["antenv.axon_hooks"] = mod
    # artifact upload needs egress this sandbox doesn't have
    import concourse.bass_utils as _bu

    _bu.upload_artifacts = lambda d: "local://" + str(d)


def _classify(seg):
    """Union-over-batches tile classification from segment_ids.

    Returns (cumsums [B,S], per-qb visible k-chunk lists, boundary index,
    q-range maps). Element (k, q) is visible iff cs[k] <= cs[q]; cs is
    non-decreasing, so visibility per (kc, qb) tile is a q-suffix:
    q0 = first col with any visible element (union over batches),
    q1 = first col from which every batch sees the whole chunk.
    Cols < q0 need no compute at all; cols in [q0, q1) need the 0/1 mask.
    """
    cs = np.cumsum(np.asarray(seg, np.int64), axis=1)
    vis_lists = [[] for _ in range(NQB)]
    bnd_index = {}
    q0map = {}
    q1map = {}
    for qb in range(NQB):
        for kc in range(NKC):
            any_computed = False
            all_full_vis = True
            q0u, q1u = QB, 0
            for b in range(B):
                c = cs[b]
                full_mask = c[kc * KC] > c[qb * QB + QB - 1]
                full_vis = c[kc * KC + KC - 1] <= c[qb * QB]
                if not full_mask:
                    any_computed = True
                if not full_vis:
                    all_full_vis = False
                qcs = c[qb * QB : (qb + 1) * QB]
                anyv = np.nonzero(qcs >= c[kc * KC])[0]
                fullv = np.nonzero(qcs >= c[kc * KC + KC - 1])[0]
                q0u = min(q0u, int(anyv[0]) if len(anyv) else QB)
                q1u = max(q1u, int(fullv[0]) if len(fullv) else QB)
            if any_computed:
                vis_lists[qb].append(kc)
                if not all_full_vis:
                    bnd_index[(kc, qb)] = len(bnd_index)
                    q0map[(kc, qb)] = (q0u // 16) * 16
                    q1map[(kc, qb)] = min(QB, ((q1u + 15) // 16) * 16)
                else:
                    q0map[(kc, qb)] = 0
                    q1map[(kc, qb)] = 0
    return cs, vis_lists, bnd_index, (q0map, q1map)


def _build_program(vis_lists, bnd_index, qmaps):
    nc = bacc.Bacc()
    q0map, q1map = qmaps
    n_bnd = max(len(bnd_index), 1)
    # qb=0 queries attend very few keys, so fp8 ctx quantization noise does
    # not average out there (worst ~6e-2 rel vs ~3e-3 for qb>=1). Keep a
    # small bf16 copy of the v chunks qb=0 needs and run its ctx in bf16.
    N0 = len(vis_lists[0])
    VW16 = HPC * (D + 1)

    xT_d = nc.declare_dram_parameter("xT", [128, FKC * S], BF16, isOutput=False)
    wq_d = nc.declare_dram_parameter("wq", [128, FKC * HD], BF16, isOutput=False)
    wk_d = nc.declare_dram_parameter("wk", [128, FKC * HD], BF16, isOutput=False)
    wv_d = nc.declare_dram_parameter("wv", [128, FKC * HD], BF16, isOutput=False)
    bqk_d = nc.declare_dram_parameter("bqk", [128, 2 * MC], F32, isOutput=False)
    bvb_d = nc.declare_dram_parameter("bvb", [128, HD], F32, isOutput=False)
    csb_d = nc.declare_dram_parameter("cs_bcast", [128, S], F32, isOutput=False)
    csp_d = nc.declare_dram_parameter("cs_part", [128, NKC], F32, isOutput=False)
    out_d = nc.declare_dram_parameter("ctxT", [MC * 128, S], F32, isOutput=True)

    with ExitStack() as ctx:
        tc = ctx.enter_context(tile.TileContext(nc))
        persist = ctx.enter_context(tc.tile_pool(name="persist", bufs=1))

        qt = persist.tile([128, MC * S], BF16)
        kt = persist.tile([128, MC * S], BF16)
        v = persist.tile([128, NKC * VW], FP8)
        vb16 = persist.tile([128, N0 * VW16], BF16)
        ctxt = persist.tile([128, MC * S], F32)
        msk = persist.tile([128, n_bnd * QB], FP8)
        mskb = persist.tile([128, n_bnd * QB], BF16)
        cs_b = persist.tile([128, S], F32)
        cs_p = persist.tile([128, NKC], F32)
        bqk_sb = persist.tile([128, 2 * MC], F32)
        bv_sb = persist.tile([128, HD], F32)
        warmsrc = persist.tile([128, 640], BF16)
        nc.vector.memset(warmsrc, 0.0)
        nc.gpsimd.load_library(library_config.attn)  # partition_broadcast ucode
        nc.sync.dma_start(out=cs_b, in_=csb_d[:])
        nc.sync.dma_start(out=cs_p, in_=csp_d[:])
        nc.sync.dma_start(out=bqk_sb, in_=bqk_d[:])
        nc.sync.dma_start(out=bv_sb, in_=bvb_d[:])

        # 0/1 masks for boundary tiles, shared by all 6 heads of this core.
        # fp8 copy for the qb>=1 fp8 ctx path, bf16 copy for the qb=0 path.
        for (kc, qb), bi in bnd_index.items():
            dstm = mskb if qb == 0 else msk
            nc.vector.tensor_scalar(
                out=dstm[:, bi * QB : (bi + 1) * QB],
                in0=cs_b[:, qb * QB : (qb + 1) * QB],
                scalar1=cs_p[:, kc : kc + 1],
                scalar2=None,
                op0=mybir.AluOpType.is_ge,
            )

        with (
            tc.tile_pool(name="ld", bufs=1) as ld,
            tc.tile_pool(name="pps", bufs=2, space="PSUM") as pps,
            tc.tile_pool(name="scps", bufs=2, space="PSUM") as scps,
            tc.tile_pool(name="ctxps", bufs=2, space="PSUM") as ctxps,
            tc.tile_pool(name="expp", bufs=3) as expp,
            tc.tile_pool(name="lpool", bufs=4) as lpool,
        ):
            # wq/wk are laid out mc-major on the host so the mc=0 slices the
            # prelude needs land first; xt lands per feature-chunk so the
            # first projection matmuls start as soon as chunk 0 arrives.
            MB = FKC * 128  # one mc block of wq/wk columns
            xt = ld.tile([128, FKC * S], BF16)
            wq_sb = ld.tile([128, MC * MB], BF16)
            wk_sb = ld.tile([128, MC * MB], BF16)
            wv_sb = ld.tile([128, FKC * HD], BF16)
            # Startup-critical tensors go in halves: one dma_start lands on a
            # single HW queue, so splitting buys queue parallelism.
            hm = MB // 2
            nc.sync.dma_start(out=wq_sb[:, 0:hm], in_=wq_d[:, 0:hm])
            nc.sync.dma_start(out=wq_sb[:, hm:MB], in_=wq_d[:, hm:MB])
            nc.sync.dma_start(out=wk_sb[:, 0:hm], in_=wk_d[:, 0:hm])
            nc.sync.dma_start(out=wk_sb[:, hm:MB], in_=wk_d[:, hm:MB])
            for kc in range(FKC):
                h2 = kc * S + S // 2
                nc.sync.dma_start(out=xt[:, kc * S : h2], in_=xT_d[:, kc * S : h2])
                nc.sync.dma_start(out=xt[:, h2 : (kc + 1) * S], in_=xT_d[:, h2 : (kc + 1) * S])
            nc.sync.dma_start(out=wv_sb, in_=wv_d[:])
            nc.sync.dma_start(out=wq_sb[:, MB:], in_=wq_d[:, MB:])
            nc.sync.dma_start(out=wk_sb[:, MB:], in_=wk_d[:, MB:])

            v4 = v.rearrange("p (s h e) -> p s h e", h=HPC, e=VE)
            v_ones = v4[:, :, :, D : D + 1]
            nc.vector.memset(v_ones, 1.0)
            vb4 = vb16.rearrange("p (s h e) -> p s h e", h=HPC, e=D + 1)
            nc.vector.memset(vb4[:, :, :, D : D + 1], 1.0)

            # --- projection quanta: one PSUM bank accumulated over the 6
            # feature chunks, drained on DVE (ACT is the kernel bottleneck).
            def qk_quantum(pi, mc, nb, drain_on_act=False):
                w_sb = wq_sb if pi == 0 else wk_sb
                out_sb = qt if pi == 0 else kt
                ps = pps.tile([128, QB], F32, tag="proj", name="psqk")
                for kc in range(FKC):
                    nc.tensor.matmul(
                        ps,
                        lhsT=w_sb[:, mc * MB + kc * 128 : mc * MB + kc * 128 + 128],
                        rhs=xt[:, kc * S + nb * QB : kc * S + (nb + 1) * QB],
                        start=(kc == 0),
                        stop=(kc == FKC - 1),
                    )
                if drain_on_act:
                    # leisure-popped quanta (consumed much later): the drain
                    # doubles as filler for gaps in the exp dependency chain.
                    nc.scalar.activation(
                        out=out_sb[:, mc * S + nb * QB : mc * S + (nb + 1) * QB],
                        in_=ps,
                        func=mybir.ActivationFunctionType.Identity,
                        bias=bqk_sb[:, pi * MC + mc : pi * MC + mc + 1],
                        scale=1.0,
                    )
                else:
                    # demanded quanta (consumed within a few groups): drain on
                    # DVE so the result is not stuck behind queued exps in
                    # the ACT FIFO.
                    nc.vector.tensor_scalar_add(
                        out_sb[:, mc * S + nb * QB : mc * S + (nb + 1) * QB],
                        ps,
                        bqk_sb[:, pi * MC + mc : pi * MC + mc + 1],
                    )

            def v_quantum(sc):
                ps = pps.tile([128, HD], F32, tag="proj", name="psv")
                for kc in range(FKC):
                    nc.tensor.matmul(
                        ps,
                        lhsT=xt[:, kc * S + sc * KC : kc * S + sc * KC + KC],
                        rhs=wv_sb[:, kc * HD : (kc + 1) * HD],
                        start=(kc == 0),
                        stop=(kc == FKC - 1),
                    )
                nc.vector.tensor_add(
                    v4[:, sc, :, 0:D],
                    ps.rearrange("p (h e) -> p h e", e=D),
                    bv_sb.rearrange("p (h e) -> p h e", e=D),
                )
                if sc < N0:
                    nc.vector.tensor_add(
                        vb4[:, sc, :, 0:D],
                        ps.rearrange("p (h e) -> p h e", e=D),
                        bv_sb.rearrange("p (h e) -> p h e", e=D),
                    )

            # Projection work is emitted as PE filler spread across the
            # attention loop: emission order defines both the dependency DAG
            # and the scheduler priority, so each quantum must be emitted
            # before its first consumer (require(), with a 2-group lookahead)
            # and the slack is spread evenly (leisure pops) to keep PE duty
            # high everywhere — otherwise the HAM clock gate re-throttles
            # the PE to 1.2 GHz and every matmul doubles in cost.
            emitted = set()

            def emit_quantum(fq, drain_on_act=False):
                if fq in emitted:
                    return
                emitted.add(fq)
                if fq[0] == "v":
                    v_quantum(fq[1])
                else:
                    qk_quantum(fq[1], fq[2], fq[3], drain_on_act)

            filler = []
            for nb in range(1, NQB):
                filler.append(("qk", 0, 0, nb))
            for nb in range(2, NQB):
                filler.append(("qk", 1, 0, nb))
            for sc in range(N0, NKC):
                filler.append(("v", sc))
            for mc in (1, 2):
                for pi in range(2):
                    for nb in range(NQB):
                        filler.append(("qk", pi, mc, nb))

            demand_q = []

            def require(fq):
                # hard: the consumer is being emitted right now
                if fq in emitted:
                    return
                if fq in filler:
                    filler.remove(fq)
                if fq in demand_q:
                    demand_q.remove(fq)
                emit_quantum(fq)

            def require_soon(fq):
                # soft: needed within the next few groups; emitted by the
                # per-site popper at a capped one-quantum-per-site rate so
                # the PE work between consecutive scores groups never
                # exceeds the exp time (else ACT starves).
                if fq in emitted or fq in demand_q:
                    return
                if fq in filler:
                    filler.remove(fq)
                demand_q.append(fq)

            def warm_dummy():
                # Throwaway matmul that keeps the PE array active when no
                # real filler is left: sub-threshold PE duty makes the HAM
                # clock gate drop the PE to 1.2 GHz, which costs far more
                # than these extra 213ns matmuls.
                ps = pps.tile([128, QB], F32, tag="proj", name="warm")
                nc.tensor.matmul(
                    ps, lhsT=warmsrc[:, 0:128], rhs=warmsrc[:, 128:128 + QB],
                    start=True, stop=True,
                )



            def leisure_pop():
                if demand_q:
                    emit_quantum(demand_q.pop(0))
                elif filler:
                    emit_quantum(filler.pop(0), drain_on_act=True)
                else:
                    for _ in range(4):
                        warm_dummy()

            def group_needs(hp, qb, g):
                needs = [("qk", 0, hp, qb)]
                nb_hi = (g[-1] * KC + KC - 1) // QB
                for nb in range(nb_hi + 1):
                    needs.append(("qk", 1, hp, nb))
                for kc in g:
                    needs.append(("v", kc))
                return needs

            # Prelude: what the first two groups consume (the rest of the
            # v chunks for qb=0 are demand-pulled by the slot lookahead).
            emit_quantum(("qk", 0, 0, 0))
            emit_quantum(("qk", 1, 0, 0))
            for sc in range(2):
                emit_quantum(("v", sc))
            for sc in range(2, N0):
                filler.insert(sc - 2, ("v", sc))

            # --- attention ---
            def emit_scores_group(hp, qb, g, gq0, fp8_path):
                mcq = hp
                sps = {}
                esb = {}
                for par in range(2):
                    sps[par] = scps.tile(
                        [128, ACT_GROUP * QB], F32, tag="sps", name=f"sps{par}"
                    )
                    esb[par] = expp.tile(
                        [128, ACT_GROUP * QB],
                        FP8 if fp8_path else BF16,
                        tag="esb8" if fp8_path else "esb16",
                        name=f"esb{par}",
                    )
                for j, kc in enumerate(g):
                    for par in range(2):
                        po = par * 64
                        nc.tensor.matmul(
                            sps[par][:, j * QB + gq0 : (j + 1) * QB],
                            lhsT=kt[po : po + 64, mcq * S + kc * KC : mcq * S + kc * KC + KC],
                            rhs=qt[po : po + 64, mcq * S + qb * QB + gq0 : mcq * S + (qb + 1) * QB],
                            start=True,
                            stop=True,
                        )
                for par in range(2):
                    if len(g) == 2:
                        src = sps[par].rearrange("p (j q) -> p j q", j=2)[:, :, gq0:]
                        dst = esb[par].rearrange("p (j q) -> p j q", j=2)[:, :, gq0:]
                    else:
                        src = sps[par][:, gq0:QB]
                        dst = esb[par][:, gq0:QB]
                    nc.scalar.activation(
                        out=dst,
                        in_=src,
                        func=mybir.ActivationFunctionType.Exp,
                        scale=1.0 / float(np.sqrt(np.float32(D))),
                    )
                for j, kc in enumerate(g):
                    bi = bnd_index.get((kc, qb))
                    if bi is None:
                        continue
                    q1 = q1map[(kc, qb)]
                    srcm = msk if fp8_path else mskb
                    for par in range(2):
                        nc.vector.tensor_mul(
                            esb[par][:, j * QB + gq0 : j * QB + q1],
                            esb[par][:, j * QB + gq0 : j * QB + q1],
                            srcm[:, bi * QB + gq0 : bi * QB + q1],
                        )
                return esb

            def emit_ctx_group(hp, qb, g, gq0, fp8_path, esb, cps, unit, n_units):
                if fp8_path:
                    for par in range(2):
                        h = 2 * hp + par
                        if len(g) == 2:
                            nc.tensor.matmul(
                                cps[par][:, gq0:],
                                lhsT=v4[:, g[0] : g[0] + 2, h, 0 : D + 1],
                                rhs=esb[par].rearrange("p (j q) -> p j q", j=2)[:, :, gq0:],
                                start=(unit == 0),
                                stop=(unit == n_units - 1),
                                perf_mode=mybir.MatmulPerfMode.DoubleRow,
                            )
                        else:
                            nc.tensor.matmul(
                                cps[par][:, gq0:],
                                lhsT=v4[:, g[0], h, 0 : D + 1],
                                rhs=esb[par][:, gq0:QB],
                                start=(unit == 0),
                                stop=(unit == n_units - 1),
                            )
                    return unit + 1
                for j, kc in enumerate(g):
                    for par in range(2):
                        h = 2 * hp + par
                        nc.tensor.matmul(
                            cps[par][:, gq0:],
                            lhsT=vb16[:, kc * VW16 + h * (D + 1) : kc * VW16 + (h + 1) * (D + 1)],
                            rhs=esb[par][:, j * QB + gq0 : (j + 1) * QB],
                            start=(unit + j == 0),
                            stop=(unit + j == n_units - 1),
                        )
                return unit + len(g)

            all_slots = []
            for hp in range(HPC // 2):
                for qb in range(NQB):
                    vis = vis_lists[qb]
                    gs = [vis[i : i + ACT_GROUP] for i in range(0, len(vis), ACT_GROUP)]
                    for g in gs:
                        all_slots.append((hp, qb, g))
            si = 0
            for fut in all_slots[0:2]:
                for need in group_needs(*fut):
                    require(need)

            def emit_drain(hp_d, qb_d, cps_d):
                # drain: 1/l via fast reciprocal, broadcast over the 64
                # d-rows on the (idle) GPSIMD engine, then one fused
                # DVE multiply PSUM*bcast -> ctxt. Nothing lands in the
                # PE instruction stream, so matmuls never stall on it.
                # Called AFTER the next q-block's first scores group is
                # emitted, so the drain latency never gaps the exp stream.
                for par in range(2):
                    po = par * 64
                    # NB: the custom-DVE reciprocal misreads on HW when the
                    # input base partition differs from the output's, so
                    # stage l on partition 0 with a standard copy first.
                    lt = lpool.tile([1, QB], F32, tag="lt", name="lt")
                    lr = lpool.tile([1, QB], F32, tag="lr", name="lr")
                    bc = lpool.tile([64, QB], F32, tag="bc", name="bc")
                    nc.vector.tensor_copy(lt, cps_d[par][64:65, :])
                    nc.vector.reciprocal_approx_fast(out=lr, in_=lt)
                    nc.gpsimd.partition_broadcast(bc, lr)
                    nc.vector.tensor_mul(
                        ctxt[po : po + 64, hp_d * S + qb_d * QB : hp_d * S + (qb_d + 1) * QB],
                        cps_d[par][0:64, :],
                        bc,
                    )
                nc.sync.dma_start(
                    out=out_d[hp_d * 128 : (hp_d + 1) * 128, qb_d * QB : (qb_d + 1) * QB],
                    in_=ctxt[:, hp_d * S + qb_d * QB : hp_d * S + (qb_d + 1) * QB],
                )

            pending_drain = None
            for hp in range(HPC // 2):
                for qb in range(NQB):
                    vis = vis_lists[qb]
                    fp8_path = qb != 0
                    groups = [vis[i : i + ACT_GROUP] for i in range(0, len(vis), ACT_GROUP)]
                    n_units = len(groups) if fp8_path else len(vis)
                    cps = {}
                    for par in range(2):
                        cps[par] = ctxps.tile([65, QB], F32, tag="cps", name=f"cps{par}")
                    unit = 0
                    prev = None
                    for gi, g in enumerate(groups):
                        for need in group_needs(hp, qb, g):
                            require(need)
                        gq0 = min(q0map[(kc, qb)] for kc in g)
                        esb = emit_scores_group(hp, qb, g, gq0, fp8_path)
                        if gi == 0 and pending_drain is not None:
                            emit_drain(*pending_drain)
                            pending_drain = None
                        for fut in all_slots[si + 1 : si + 4]:
                            for need in group_needs(*fut):
                                require_soon(need)
                        leisure_pop()
                        si += 1
                        if prev is not None:
                            unit = emit_ctx_group(hp, qb, *prev, cps, unit, n_units)
                        prev = (g, gq0, fp8_path, esb)
                    leisure_pop()
                    unit = emit_ctx_group(hp, qb, *prev, cps, unit, n_units)
                    pending_drain = (hp, qb, cps)
            emit_drain(*pending_drain)

    nc.finalize()
    return nc


def _core_inputs(x, segment_ids, Wq, bq, Wk, bk, Wv, bv, cs, core):
    b, h0 = core // 2, HPC * (core % 2)
    cols = slice(h0 * D, (h0 + HPC) * D)
    xT = np.ascontiguousarray(x[b].T)  # [768, 2048]
    xT_s = (
        xT.reshape(FKC, 128, S).transpose(1, 0, 2).reshape(128, FKC * S)
    ).astype(ml_dtypes.bfloat16)

    def wprep(Wm, mc_major):
        ws = Wm[:, cols]  # [768, 384]
        if mc_major:  # [128, (mc, kc, 128)] so per-mc DMA chunks are contiguous
            arr = ws.reshape(FKC, 128, MC, 128).transpose(1, 2, 0, 3)
        else:  # [128, (kc, col)]
            arr = ws.reshape(FKC, 128, HD).transpose(1, 0, 2)
        return np.ascontiguousarray(arr.reshape(128, FKC * HD)).astype(
            ml_dtypes.bfloat16
        )

    bq_s = np.ascontiguousarray(bq[cols].reshape(MC, 128).T)
    bk_s = np.ascontiguousarray(bk[cols].reshape(MC, 128).T)
    bqk = np.concatenate([bq_s, bk_s], axis=1)  # [128, 6]
    bvb = np.ascontiguousarray(np.broadcast_to(bv[cols], (128, HD)))
    csf = cs[b].astype(np.float32)
    cs_bcast = np.ascontiguousarray(np.broadcast_to(csf, (128, S)))
    cs_part = np.ascontiguousarray(csf.reshape(NKC, KC).T)
    return {
        "xT": np.ascontiguousarray(xT_s),
        "wq": wprep(Wq, True),
        "wk": wprep(Wk, True),
        "wv": wprep(Wv, False),
        "bqk": np.ascontiguousarray(bqk),
        "bvb": bvb,
        "cs_bcast": cs_bcast,
        "cs_part": cs_part,
    }


def kernel(x, segment_ids, Wq, bq, Wk, bk, Wv, bv):
    global LAST_RESULTS
    x = np.asarray(x, np.float32)
    segment_ids = np.asarray(segment_ids)
    Wq, bq = np.asarray(Wq, np.float32), np.asarray(bq, np.float32)
    Wk, bk = np.asarray(Wk, np.float32), np.asarray(bk, np.float32)
    Wv, bv = np.asarray(Wv, np.float32), np.asarray(bv, np.float32)

    cs, vis_lists, bnd_index, qmaps = _classify(segment_ids)
    nc = _build_program(vis_lists, bnd_index, qmaps)
    in_maps = [
        _core_inputs(x, segment_ids, Wq, bq, Wk, bk, Wv, bv, cs, c)
        for c in range(NCORES)
    ]
    if TRACE:
        _ensure_ntff_hook()
    res = run_bass_kernel_spmd(nc, in_maps, list(range(NCORES)), trace=TRACE)
    LAST_RESULTS = res

    out = np.empty((B, S, W), np.float32)
    for c in range(NCORES):
        b, h0 = c // 2, HPC * (c % 2)
        out[b, :, h0 * D : (h0 + HPC) * D] = res.results[c]["ctxT"].T
    return out
